# revision 2
# baseline (speedup 1.0000x reference)
"""Trainium2 Bass kernel for nn_AllAmplitude: per-event |amplitude|^2 sum.

Strategy: all params-derived complex couplings are computed on the host and
baked into the compiled kernel as immediate constants. The per-event math is
a straight-line elementwise program (DVE + ACT engines), evaluated on
[128 x FD] tiles; events are sharded across 8 NeuronCores.
"""
import os
import numpy as np
from math import factorial as _f, sqrt as _msqrt
from contextlib import ExitStack

# ---------------------------------------------------------------- constants
JA, JB, JC, JD = 1, 1, 0, 1
PA, PB, PC, PD = -1, -1, -1, -1
M0_B, M0_C, M0_D = 2.01026, 0.13957061, 2.00685
RES = [("Zc_4025", -1, 4.026, 0.025, 1, 1),
       ("D2_2460", 21, 2.4607, 0.0475, 2, 1),
       ("D1_2420", 121, 2.4232, 0.025, 1, 1)]
N_FULL = 2000000
N_CORES = 8
PER_CORE = N_FULL // N_CORES            # 250000
COLS = 1956                             # 128*1956 = 250368 >= 250000
PER_CORE_PAD = 128 * COLS
COL_TILES = [(0, 512), (512, 512), (1024, 512), (1536, 420)]
SQ2 = float(np.sqrt(2.0))
HALF_PI = float(np.pi / 2)

INPUT_NAMES = ["m_BC", "m_BD", "m_CD", "cosTheta_BC", "cosTheta_B_BC",
               "phi_BC", "phi_B_BC", "cosTheta_BD", "cosTheta_D_BD",
               "phi_D_BD", "cosTheta_CD", "cosTheta_C_CD", "phi_CD",
               "phi_C_CD", "cosTheta1", "cosTheta2", "phi1", "phi2"]


def cg_coef(j1, j2, m1, m2, j, m):
    if m1 + m2 != m or j < abs(j1 - j2) or j > j1 + j2:
        return 0.0
    if abs(m1) > j1 or abs(m2) > j2 or abs(m) > j:
        return 0.0
    pref = _msqrt((2 * j + 1) * _f(j1 + j2 - j) * _f(j + j1 - j2) * _f(j + j2 - j1) / _f(j1 + j2 + j + 1))
    pref *= _msqrt(_f(j + m) * _f(j - m) * _f(j1 - m1) * _f(j1 + m1) * _f(j2 - m2) * _f(j2 + m2))
    s = 0.0
    for k in range(max(0, j2 - j - m1, j1 + m2 - j), min(j1 + j2 - j, j1 - m1, j2 + m2) + 1):
        s += (-1) ** k / (_f(k) * _f(j1 + j2 - j - k) * _f(j1 - m1 - k) * _f(j2 + m2 - k) * _f(j - j2 + m1 + k) * _f(j - j1 - m2 + k))
    return pref * s


def ls_list(ja, jb, jc, pa, pb, pc):
    dl = 0 if pa * pb * pc == 1 else 1
    out = []
    for s in range(abs(jb - jc), jb + jc + 1):
        for l in range(abs(ja - s), ja + s + 1):
            if l % 2 == dl:
                out.append((l, s))
    return out


def _build_layout():
    layout, off = [], 0
    for (_, chain, m0, g0, J, Par) in RES:
        jc, jd, je = (0, 1, 1) if chain < 0 else ((1, 1, 0) if chain < 100 else (1, 0, 1))
        ls0 = ls_list(JA, J, jc, PA, Par, -1)
        ls1 = ls_list(J, jd, je, Par, -1, -1)
        layout.append(((off, ls0), (off + len(ls0), ls1)))
        off += len(ls0) + len(ls1)
    return layout, off


LAYOUT, NCOEF = _build_layout()


def H_ls_val(params, entry, ja, jb, jc, lb, lc):
    off, ls = entry
    tot = 0.0 + 0.0j
    nz = False
    for idx, (l, s) in enumerate(ls):
        c = cg_coef(jb, jc, lb, -lc, s, lb - lc) * cg_coef(l, s, 0, lb - lc, ja, lb - lc)
        if c != 0.0:
            nz = True
            tot = tot + c * (params[off + idx, 0] + 1j * params[off + idx, 1])
    return tot if nz else None


def wigner_d_val(j, m1, m2, c):
    if abs(m1) > j or abs(m2) > j:
        return None
    ch = np.sqrt((1.0 + c) * 0.5)
    sh = np.sqrt((1.0 - c) * 0.5)
    pref = _msqrt(_f(j + m1) * _f(j - m1) * _f(j + m2) * _f(j - m2))
    out = 0.0
    for s in range(max(0, m2 - m1), min(j - m1, j + m2) + 1):
        coef = pref * (-1) ** (m1 - m2 + s) / (_f(j + m2 - s) * _f(s) * _f(m1 - m2 + s) * _f(j - m1 - s))
        out = out + coef * ch ** (2 * j + m2 - m1 - 2 * s) * sh ** (m1 - m2 + 2 * s)
    return out


def getp_const(M0, M1, M2):
    s, d = M1 + M2, M1 - M2
    p = (M0 - s) * (M0 + s) * (M0 - d) * (M0 + d)
    return _msqrt(p) / (2.0 * M0)


def fit_d2_poly(lBC, lB2):
    """d2_{lBC,lB2}(c) = a0 + a1*c + a2*c^2 + b0*s + b1*s*c (exact for j=2)."""
    cs = np.linspace(-0.95, 0.95, 21)
    ss = np.sqrt(1 - cs**2)
    A = np.stack([np.ones_like(cs), cs, cs**2, ss, ss * cs], 1)
    y = np.array([wigner_d_val(2, lBC, lB2, c) for c in cs], float)
    coef, *_ = np.linalg.lstsq(A, y, rcond=None)
    assert np.abs(A @ coef - y).max() < 1e-10, (lBC, lB2)
    coef[np.abs(coef) < 1e-12] = 0.0
    return coef


# ---------------------------------------------------------------- expr DAG
MULT, ADD, SUB, MAX = "mult", "add", "subtract", "max"


class Prog:
    def __init__(self):
        self.nodes = []
        self.cache = {}

    def _emit(self, key):
        if key in self.cache:
            return self.cache[key]
        idx = len(self.nodes)
        self.nodes.append(key)
        self.cache[key] = idx
        return idx

    def inp(self, name):
        return self._emit(("IN", name))

    def tt(self, a, b, alu=MULT):
        if alu in (MULT, ADD):
            a, b = min(a, b), max(a, b)
        return self._emit(("TT", a, b, alu))

    def ts(self, a, s1, op0=MULT, s2=None, op1=None):
        s1 = float(s1)
        if s2 is not None:
            s2 = float(s2)
        if op0 == MULT and s1 == 1.0 and s2 is None:
            return a
        return self._emit(("TS", a, s1, op0, s2, op1))

    def stt(self, a, s, b, op0=MULT, op1=ADD):
        s = float(s)
        if op0 == MULT and s == 1.0:
            return self.tt(a, b, op1)
        return self._emit(("STT", a, s, b, op0, op1))

    def act(self, func, a, scale=1.0, bias=0.0):
        return self._emit(("ACT", func, a, float(scale), float(bias)))

    def rec(self, a):
        return self._emit(("REC", a))

    def deps(self, i):
        nd = self.nodes[i]
        op = nd[0]
        if op == "TT":
            return [nd[1], nd[2]]
        if op == "TS":
            return [nd[1]]
        if op == "STT":
            return [nd[1], nd[3]]
        if op == "ACT":
            return [nd[2]]
        if op == "REC":
            return [nd[1]]
        return []

    def evaluate(self, inputs, outputs, dtype=np.float64):
        vals = [None] * len(self.nodes)
        for i, nd in enumerate(self.nodes):
            op = nd[0]
            if op == "IN":
                vals[i] = np.asarray(inputs[nd[1]], dtype)
            elif op == "TT":
                _, a, b, alu = nd
                x, y = vals[a], vals[b]
                vals[i] = {MULT: lambda: x * y, ADD: lambda: x + y,
                           SUB: lambda: x - y,
                           MAX: lambda: np.maximum(x, y)}[alu]().astype(dtype)
            elif op == "TS":
                _, a, s1, op0, s2, op1 = nd
                x = vals[a]
                f = {MULT: lambda v, s: v * s, ADD: lambda v, s: v + s,
                     SUB: lambda v, s: v - s}
                r = f[op0](x, dtype(s1))
                if s2 is not None:
                    r = f[op1](r, dtype(s2))
                vals[i] = r.astype(dtype)
            elif op == "STT":
                _, a, s, b, op0, op1 = nd
                x, y = vals[a], vals[b]
                r = {MULT: lambda: x * dtype(s), ADD: lambda: x + dtype(s)}[op0]()
                r = {ADD: lambda: r + y, SUB: lambda: r - y,
                     MULT: lambda: r * y}[op1]()
                vals[i] = r.astype(dtype)
            elif op == "ACT":
                _, func, a, scale, bias = nd
                x = vals[a] * dtype(scale) + dtype(bias)
                if func == "Sin":
                    vals[i] = np.sin(x).astype(dtype)
                elif func == "Sqrt":
                    vals[i] = np.sqrt(np.maximum(x, 0)).astype(dtype)
                else:
                    raise ValueError(func)
            elif op == "REC":
                vals[i] = (dtype(1.0) / vals[nd[1]]).astype(dtype)
        return [vals[o] for o in outputs]

    def live_set(self, outputs):
        live = set()
        stack = list(outputs)
        while stack:
            i = stack.pop()
            if i in live:
                continue
            live.add(i)
            stack += self.deps(i)
        return live


# ---------------------------------------------------------------- program
def build_program(params):
    p = Prog()
    P = np.asarray(params, np.float64)
    H = H_ls_val
    L = LAYOUT
    H1_0 = {l: H(P, L[0][0], 1, 1, 0, l, 0) for l in (-1, 0, 1)}
    H2_0 = {(lD, lB): H(P, L[0][1], 1, 1, 1, lD, lB) for lD in (-1, 0, 1) for lB in (-1, 0, 1)}
    H1_1 = {(lBC, lD): H(P, L[1][0], 1, 2, 1, lBC, lD) for lBC in range(-2, 3) for lD in (-1, 0, 1)}
    H2_1 = {l: H(P, L[1][1], 2, 1, 0, l, 0) for l in (-1, 0, 1)}
    H1_2 = {(lCD, lB): H(P, L[2][0], 1, 1, 1, lCD, lB) for lCD in (-1, 0, 1) for lB in (-1, 0, 1)}
    H2_2 = {l: H(P, L[2][1], 1, 0, 1, 0, l) for l in (-1, 0, 1)}
    z = lambda x: 0j if x is None else x

    IN = {k: p.inp(k) for k in INPUT_NAMES}

    def bw_nodes(m_idx, m0, g0, M1, M2):
        s0, d0 = M1 + M2, M1 - M2
        q0 = getp_const(m0, M1, M2)
        K = g0 * m0 * m0 / (2.0 * q0)
        m2 = p.tt(m_idx, m_idx)
        t = p.ts(m2, -s0 * s0, ADD)
        u = p.ts(m2, -d0 * d0, ADD)
        pp = p.tt(t, u)
        g = p.act("Sqrt", pp, scale=K * K)
        dre = p.ts(m2, -1.0, MULT, m0 * m0, ADD)
        e = p.tt(m2, dre)
        e2 = p.tt(e, e)
        g2 = p.tt(g, g)
        dab = p.tt(e2, g2, ADD)
        dinv = p.rec(dab)
        w = p.tt(g, dinv)
        return (p.tt(w, e), p.tt(w, g))

    bw0 = bw_nodes(IN["m_BD"], 4.026, 0.025, M0_B, M0_D)
    bw1 = bw_nodes(IN["m_BC"], 2.4607, 0.0475, M0_B, M0_C)
    bw2 = bw_nodes(IN["m_CD"], 2.4232, 0.025, M0_C, M0_D)

    def sine(c_idx):
        c2 = p.tt(c_idx, c_idx)
        return p.act("Sqrt", c2, scale=-1.0, bias=1.0), c2

    cBD = IN["cosTheta_BD"];   sBD, _ = sine(cBD)
    cDD = IN["cosTheta_D_BD"]; sDD, _ = sine(cDD)
    cBCa = IN["cosTheta_BC"];  sBCa, _ = sine(cBCa)
    cBB = IN["cosTheta_B_BC"]; sBB, c2BB = sine(cBB)
    cCDa = IN["cosTheta_CD"];  sCDa, _ = sine(cCDa)
    cCC = IN["cosTheta_C_CD"]; sCC, _ = sine(cCC)
    ct1 = IN["cosTheta1"];     st1, _ = sine(ct1)
    ct2 = IN["cosTheta2"];     st2, _ = sine(ct2)

    # phases: sin(x) direct; cos(x) = Sin(pi/2 - |x|) (range-safe);
    # phi sums via addition formulas.
    def sincos(x_idx):
        s = p.act("Sin", x_idx)
        neg = p.ts(x_idx, -1.0)
        ax = p.tt(x_idx, neg, MAX)
        c = p.act("Sin", ax, scale=-1.0, bias=HALF_PI)
        return s, c

    sD_, cD_ = sincos(IN["phi_D_BD"])
    sB1, cB1 = sincos(IN["phi_BC"])
    sC1, cC1 = sincos(IN["phi_CD"])
    sp1, cp1 = sincos(IN["phi1"])
    sbb, cbb = sincos(IN["phi_B_BC"])
    sp2, cp2 = sincos(IN["phi2"])
    scc, ccc = sincos(IN["phi_C_CD"])
    sA = p.tt(p.tt(sp1, cbb), p.tt(cp1, sbb), ADD)
    cA = p.tt(p.tt(cp1, cbb), p.tt(sp1, sbb), SUB)
    sB_ = p.tt(p.tt(sp2, ccc), p.tt(cp2, scc), ADD)
    cB_ = p.tt(p.tt(cp2, ccc), p.tt(sp2, scc), SUB)
    s2q = p.tt(sB1, sB1)
    scq = p.tt(sB1, cB1)

    def d1_vals(c_idx, s_idx):
        return {"u": p.ts(c_idx, 0.5, MULT, 0.5, ADD),
                "v": p.ts(c_idx, -0.5, MULT, 0.5, ADD),
                "w": p.ts(s_idx, 1.0 / SQ2),
                "c": c_idx}

    vBD = d1_vals(cBD, sBD)
    vDD = d1_vals(cDD, sDD)
    vBC = d1_vals(cBCa, sBCa)
    vT1 = d1_vals(ct1, st1)
    vT2 = d1_vals(ct2, st2)
    vCD = d1_vals(cCDa, sCDa)
    vCC = d1_vals(cCC, sCC)

    prodcache = {}

    def vprod(va, ka, vb, kb, tag):
        key = (tag, ka, kb)
        if key not in prodcache:
            prodcache[key] = p.tt(va[ka], vb[kb])
        return prodcache[key]

    d1row = {1: {1: ("u", 1.0), 0: ("w", -1.0), -1: ("v", 1.0)},
             0: {1: ("w", 1.0), 0: ("c", 1.0), -1: ("w", -1.0)},
             -1: {1: ("v", 1.0), 0: ("w", 1.0), -1: ("u", 1.0)}}

    # ---- res0 ----
    b0r, b0i = bw0
    q1 = p.tt(b0r, cD_); q2 = p.tt(b0i, sD_); q3 = p.tt(b0i, cD_); q4 = p.tt(b0r, sD_)
    Bc = {0: (b0r, b0i),
          1: (p.tt(q1, q2, ADD), p.tt(q3, q4, SUB)),
          -1: (p.tt(q1, q2, SUB), p.tt(q3, q4, ADD))}

    amps = {}
    for lA in (-1, 1):
        for lB in (-1, 0, 1):
            for lD in (-1, 0, 1):
                mu = lD - lB
                if abs(mu) > 1 or z(H2_0[(lD, lB)]) == 0j:
                    continue
                terms = []
                for lDB in (-1, 0, 1):
                    h = z(H1_0[lDB]) * z(H2_0[(lD, lB)])
                    if h == 0j:
                        continue
                    ka, sa = d1row[lA][lDB]
                    kb, sb = d1row[lDB][mu]
                    terms.append((h * sa * sb, vprod(vBD, ka, vDD, kb, "r0")))
                if not terms:
                    continue
                gre = gim = None
                for (cf, prod) in terms:
                    cr, ci = float(cf.real), float(cf.imag)
                    gre = p.ts(prod, cr) if gre is None else p.stt(prod, cr, gre)
                    gim = p.ts(prod, ci) if gim is None else p.stt(prod, ci, gim)
                Br, Bi = Bc[lD]
                rr = p.tt(Br, gre); ii = p.tt(Bi, gim)
                ri = p.tt(Br, gim); ir = p.tt(Bi, gre)
                amps[(lA, lB, lD)] = (p.tt(rr, ii, SUB), p.tt(ri, ir, ADD))

    # ---- res1 ----
    b1r, b1i = bw1
    r1 = p.tt(b1r, cA); r2 = p.tt(b1i, sA); r3 = p.tt(b1i, cA); r4 = p.tt(b1r, sA)
    P1 = {1: (p.tt(r1, r2, ADD), p.tt(r3, r4, SUB)),
          -1: (p.tt(r1, r2, SUB), p.tt(r3, r4, ADD))}

    cU = {}
    for lBC in (-2, -1, 1, 2):
        for lD in (-1, 0, 1):
            if abs(lBC - lD) > 1:
                continue
            h = z(H1_1[(lBC, lD)])
            if h == 0j:
                continue
            hr, hi = float(h.real), float(h.imag)
            if lBC == 1:
                re = p.stt(sB1, hi, p.ts(cB1, hr))
                im = p.stt(sB1, -hr, p.ts(cB1, hi))
            elif lBC == -1:
                re = p.stt(sB1, -hi, p.ts(cB1, hr))
                im = p.stt(sB1, hr, p.ts(cB1, hi))
            else:
                sg = 1.0 if lBC == 2 else -1.0
                re = p.stt(scq, 2 * hi * sg, p.ts(s2q, -2 * hr, MULT, hr, ADD))
                im = p.stt(scq, -2 * hr * sg, p.ts(s2q, -2 * hi, MULT, hi, ADD))
            cU[(lBC, lD)] = (re, im)

    scB = p.tt(sBB, cBB)
    d2cache = {}

    def d2val(lBC, lB2):
        key = (lBC, lB2)
        if key in d2cache:
            return d2cache[key]
        mkey = (-lBC, -lB2)
        if mkey in d2cache:
            base, sgn = d2cache[mkey]
            d2cache[key] = (base, sgn * (-1.0) ** (lBC - lB2))
            return d2cache[key]
        a0, a1, a2, b0, b1 = fit_d2_poly(lBC, lB2)
        node = None
        for (cf, src) in ((a1, cBB), (a2, c2BB), (b0, sBB), (b1, scB)):
            if cf == 0.0:
                continue
            node = p.ts(src, cf) if node is None else p.stt(src, cf, node)
        if a0 != 0.0:
            node = p.ts(node, 1.0, MULT, a0, ADD)
        d2cache[key] = (node, 1.0)
        return d2cache[key]

    W = {}
    for lA in (-1, 1):
        for lD in (-1, 0, 1):
            for lB2 in (-1, 1):
                h2 = z(H2_1[lB2])
                if h2 == 0j:
                    continue
                h2r, h2i = float(h2.real), float(h2.imag)
                ev_terms, const_terms = [], []
                for lBC in range(max(-2, lD - 1), min(2, lD + 1) + 1):
                    nu = lBC - lD
                    ka, sa = d1row[lA][nu]
                    d2n, sgn2 = d2val(lBC, lB2)
                    ddp = p.tt(vBC[ka], d2n)
                    sc = sa * sgn2
                    if lBC == 0:
                        h = z(H1_1[(lBC, lD)]) * h2
                        if h != 0j:
                            const_terms.append((h * sc, ddp))
                    elif (lBC, lD) in cU:
                        ev_terms.append((sc, cU[(lBC, lD)], ddp))

                def wsum(comp):
                    node = None
                    for (sc, (ur, ui), ddp) in ev_terms:
                        tr = p.tt(ur, ddp)
                        ti = p.tt(ui, ddp)
                        cfr = sc * (h2r if comp == "re" else h2i)
                        cfi = sc * (-h2i if comp == "re" else h2r)
                        if cfr != 0.0:
                            node = p.ts(tr, cfr) if node is None else p.stt(tr, cfr, node)
                        if cfi != 0.0:
                            node = p.ts(ti, cfi) if node is None else p.stt(ti, cfi, node)
                    for (cf0, ddp) in const_terms:
                        cf = float(cf0.real) if comp == "re" else float(cf0.imag)
                        if cf == 0.0:
                            continue
                        node = p.ts(ddp, cf) if node is None else p.stt(ddp, cf, node)
                    return node
                wre, wim = wsum("re"), wsum("im")
                if wre is None and wim is None:
                    continue
                W[(lA, lD, lB2)] = (wre, wim)

    Zc = {}
    for key, (wre, wim) in W.items():
        lA, lD, lB2 = key
        Pr, Pi = P1[lB2]
        rr = p.tt(Pr, wre); ii = p.tt(Pi, wim)
        ri = p.tt(Pr, wim); ir = p.tt(Pi, wre)
        Zc[key] = (p.tt(rr, ii, SUB), p.tt(ri, ir, ADD))

    for lA in (-1, 1):
        for lB in (-1, 0, 1):
            for lD in (-1, 0, 1):
                acc = amps.get((lA, lB, lD))
                for lB2 in (-1, 1):
                    if (lA, lD, lB2) not in Zc:
                        continue
                    zr, zi = Zc[(lA, lD, lB2)]
                    ka, sa = d1row[lB2][lB]
                    val = vT1[ka]
                    tre = p.tt(val, zr); tim = p.tt(val, zi)
                    if acc is None:
                        acc = ((p.ts(tre, sa), p.ts(tim, sa)) if sa != 1.0 else (tre, tim))
                    else:
                        op = ADD if sa > 0 else SUB
                        acc = (p.tt(acc[0], tre, op), p.tt(acc[1], tim, op))
                amps[(lA, lB, lD)] = acc

    # ---- res2 ----
    b2r, b2i = bw2
    u1 = p.tt(b2r, cB_); u2 = p.tt(b2i, sB_); u3 = p.tt(b2i, cB_); u4 = p.tt(b2r, sB_)
    P2 = {1: (p.tt(u1, u2, ADD), p.tt(u3, u4, SUB)),
          -1: (p.tt(u1, u2, SUB), p.tt(u3, u4, ADD)),
          0: bw2}

    cU2 = {}
    for lCD in (-1, 1):
        for lB in (-1, 0, 1):
            if abs(lCD - lB) > 1:
                continue
            h = z(H1_2[(lCD, lB)])
            if h == 0j:
                continue
            hr, hi = float(h.real), float(h.imag)
            if lCD == 1:
                re = p.stt(sC1, hi, p.ts(cC1, hr))
                im = p.stt(sC1, -hr, p.ts(cC1, hi))
            else:
                re = p.stt(sC1, -hi, p.ts(cC1, hr))
                im = p.stt(sC1, hr, p.ts(cC1, hi))
            cU2[(lCD, lB)] = (re, im)

    V = {}
    for lA in (-1, 1):
        for lB in (-1, 0, 1):
            for lC2 in (-1, 0, 1):
                h2 = z(H2_2[lC2])
                if h2 == 0j:
                    continue
                h2r, h2i = float(h2.real), float(h2.imag)
                ev_terms, const_terms = [], []
                for lCD in (-1, 0, 1):
                    if abs(lCD - lB) > 1:
                        continue
                    h1 = z(H1_2[(lCD, lB)])
                    if h1 is None or h1 == 0j:
                        continue
                    nu = lCD - lB
                    ka, sa = d1row[lA][nu]
                    kb, sb = d1row[lCD][lC2]
                    ddp = vprod(vCD, ka, vCC, kb, "r2")
                    if lCD == 0:
                        const_terms.append((h1 * h2 * sa * sb, ddp))
                    else:
                        ev_terms.append((sa * sb, cU2[(lCD, lB)], ddp))

                def vsum(comp):
                    node = None
                    for (sc, (ur, ui), ddp) in ev_terms:
                        tr = p.tt(ur, ddp)
                        ti = p.tt(ui, ddp)
                        cfr = sc * (h2r if comp == "re" else h2i)
                        cfi = sc * (-h2i if comp == "re" else h2r)
                        if cfr != 0.0:
                            node = p.ts(tr, cfr) if node is None else p.stt(tr, cfr, node)
                        if cfi != 0.0:
                            node = p.ts(ti, cfi) if node is None else p.stt(ti, cfi, node)
                    for (cf0, ddp) in const_terms:
                        cf = float(cf0.real) if comp == "re" else float(cf0.imag)
                        if cf == 0.0:
                            continue
                        node = p.ts(ddp, cf) if node is None else p.stt(ddp, cf, node)
                    return node
                vre, vim = vsum("re"), vsum("im")
                if vre is None and vim is None:
                    continue
                V[(lA, lB, lC2)] = (vre, vim)

    Z2 = {}
    for key, (vre, vim) in V.items():
        lA, lB, lC2 = key
        Pr, Pi = P2[lC2]
        rr = p.tt(Pr, vre); ii = p.tt(Pi, vim)
        ri = p.tt(Pr, vim); ir = p.tt(Pi, vre)
        Z2[key] = (p.tt(rr, ii, SUB), p.tt(ri, ir, ADD))

    for lA in (-1, 1):
        for lB in (-1, 0, 1):
            for lD in (-1, 0, 1):
                acc = amps.get((lA, lB, lD))
                for lC2 in (-1, 0, 1):
                    if (lA, lB, lC2) not in Z2:
                        continue
                    zr, zi = Z2[(lA, lB, lC2)]
                    ka, sa = d1row[lC2][lD]
                    val = vT2[ka]
                    tre = p.tt(val, zr); tim = p.tt(val, zi)
                    if acc is None:
                        acc = ((p.ts(tre, sa), p.ts(tim, sa)) if sa != 1.0 else (tre, tim))
                    else:
                        op = ADD if sa > 0 else SUB
                        acc = (p.tt(acc[0], tre, op), p.tt(acc[1], tim, op))
                amps[(lA, lB, lD)] = acc

    total = None
    for key in sorted(amps):
        ar, ai = amps[key]
        s1 = p.tt(ar, ar)
        total = s1 if total is None else p.tt(total, s1, ADD)
        s2 = p.tt(ai, ai)
        total = p.tt(total, s2, ADD)
    return p, total


# ---------------------------------------------------------------- codegen
def emit_bass(params):
    import concourse.bass as bass
    import concourse.tile as tile
    from concourse import bacc, mybir

    prog, out_idx = build_program(params)
    live = prog.live_set([out_idx])
    order = [i for i in range(len(prog.nodes)) if i in live]
    last_use = {}
    for i in order:
        for d in prog.deps(i):
            last_use[d] = i
    last_use[out_idx] = len(prog.nodes) + 1

    # max live for pool sizing
    alive, maxlive = set(), 0
    for i in order:
        alive.add(i)
        for d in prog.deps(i):
            if d in alive and last_use.get(d) == i:
                alive.discard(d)
        maxlive = max(maxlive, len(alive))
    nbufs = maxlive + 10
    assert nbufs * 512 * 4 <= 222 * 1024, nbufs

    nc = bacc.Bacc("TRN2", target_bir_lowering=False, debug=False,
                   num_devices=N_CORES)
    in_aps = {k: nc.dram_tensor(k, [128, COLS], mybir.dt.float32,
                                kind="ExternalInput").ap() for k in INPUT_NAMES}
    out_ap = nc.dram_tensor("total", [128, COLS], mybir.dt.float32,
                            kind="ExternalOutput").ap()

    # const AP for pi/2 activation bias
    for value in (HALF_PI,):
        key = (mybir.dt.float32, value)
        if key not in nc.const_aps.aps:
            t = nc.alloc_sbuf_tensor(f"const-f32-{value}", [128, 1], mybir.dt.float32)
            nc.gpsimd.memset(t.ap(), value)
            nc.const_aps.aps[key] = t.ap()
    nc.all_engine_barrier()

    ALU = {MULT: mybir.AluOpType.mult, ADD: mybir.AluOpType.add,
           SUB: mybir.AluOpType.subtract, MAX: mybir.AluOpType.max}
    AFUNC = {"Sin": mybir.ActivationFunctionType.Sin,
             "Sqrt": mybir.ActivationFunctionType.Sqrt}

    with tile.TileContext(nc) as tc:
        with ExitStack() as ctx:
            pool = ctx.enter_context(tc.tile_pool(name="slab", bufs=nbufs))
            for (c0, fd) in COL_TILES:
                tiles = {}
                for i in order:
                    nd = prog.nodes[i]
                    op = nd[0]
                    t = pool.tile([128, fd], mybir.dt.float32, tag="slab")
                    if op == "IN":
                        nc.sync.dma_start(t[:], in_aps[nd[1]][:, c0:c0 + fd])
                    elif op == "TT":
                        nc.vector.tensor_tensor(t[:], tiles[nd[1]][:], tiles[nd[2]][:], ALU[nd[3]])
                    elif op == "TS":
                        _, a, s1, op0, s2, op1 = nd
                        if s2 is None:
                            nc.vector.tensor_scalar(t[:], tiles[a][:], s1, None, ALU[op0])
                        else:
                            nc.vector.tensor_scalar(t[:], tiles[a][:], s1, s2, ALU[op0], ALU[op1])
                    elif op == "STT":
                        _, a, s, b, op0, op1 = nd
                        nc.vector.scalar_tensor_tensor(t[:], tiles[a][:], s, tiles[b][:], ALU[op0], ALU[op1])
                    elif op == "ACT":
                        _, func, a, scale, bias = nd
                        nc.scalar.activation(t[:], tiles[a][:], AFUNC[func], bias=bias, scale=scale)
                    elif op == "REC":
                        nc.vector.reciprocal(t[:], tiles[nd[1]][:])
                    tiles[i] = t
                    if i == out_idx:
                        nc.sync.dma_start(out_ap[:, c0:c0 + fd], t[:])
    nc.compile()
    return nc


# ---------------------------------------------------------------- kernel
def kernel(**inputs):
    params = np.asarray(inputs["params"], np.float32)
    nc = emit_bass(params)

    from concourse.bass_utils import run_bass_kernel_spmd

    in_maps = []
    for core in range(N_CORES):
        m = {}
        for k in INPUT_NAMES:
            arr = np.asarray(inputs[k], np.float32)[core * PER_CORE:(core + 1) * PER_CORE]
            pad = np.empty(PER_CORE_PAD, np.float32)
            pad[:PER_CORE] = arr
            pad[PER_CORE:] = arr[0]
            m[k] = pad.reshape(128, COLS)
        in_maps.append(m)

    trace = bool(os.environ.get("KERNEL_TRACE"))
    if trace:
        try:
            import sys, types
            from trn_agent_boot.trn_boot import _ntff_profile_via_ctypes
            hook = _ntff_profile_via_ctypes('/opt/axon/libaxon_pjrt.so')
            mmod = types.ModuleType('antenv.axon_hooks')
            mmod.get_axon_ntff_profile_hook = lambda: hook
            mmod.set_axon_ntff_profile_hook = lambda h: None
            sys.modules.setdefault('antenv.axon_hooks', mmod)
        except Exception as e:
            print("profile shim failed:", e)
            trace = False

    res = run_bass_kernel_spmd(nc, in_maps, core_ids=list(range(N_CORES)),
                               trace=trace)
    if trace and res.exec_time_ns is not None:
        print(f"HW exec time: {res.exec_time_ns} ns")

    out = np.empty(N_FULL, np.float32)
    for core in range(N_CORES):
        out[core * PER_CORE:(core + 1) * PER_CORE] = \
            res.results[core]["total"].reshape(-1)[:PER_CORE]
    return out


# revision 3
# speedup vs baseline: 1.2823x; 1.2823x over previous
"""Trainium2 Bass kernel for nn_AllAmplitude: per-event |amplitude|^2 sum.

Strategy: all params-derived complex couplings are computed on the host and
baked into the compiled kernel as immediate constants. The per-event math is
a straight-line elementwise program (DVE + ACT engines), evaluated on
[128 x FD] tiles; events are sharded across 8 NeuronCores.
"""
import os
import numpy as np
from math import factorial as _f, sqrt as _msqrt
from contextlib import ExitStack

# ---------------------------------------------------------------- constants
JA, JB, JC, JD = 1, 1, 0, 1
PA, PB, PC, PD = -1, -1, -1, -1
M0_B, M0_C, M0_D = 2.01026, 0.13957061, 2.00685
RES = [("Zc_4025", -1, 4.026, 0.025, 1, 1),
       ("D2_2460", 21, 2.4607, 0.0475, 2, 1),
       ("D1_2420", 121, 2.4232, 0.025, 1, 1)]
N_FULL = 2000000
N_CORES = 8
PER_CORE = N_FULL // N_CORES            # 250000
COLS = 1956                             # 128*1956 = 250368 >= 250000
PER_CORE_PAD = 128 * COLS
COL_TILES = [(0, 512), (512, 512), (1024, 512), (1536, 420)]
SQ2 = float(np.sqrt(2.0))
HALF_PI = float(np.pi / 2)

INPUT_NAMES = ["m_BC", "m_BD", "m_CD", "cosTheta_BC", "cosTheta_B_BC",
               "phi_BC", "phi_B_BC", "cosTheta_BD", "cosTheta_D_BD",
               "phi_D_BD", "cosTheta_CD", "cosTheta_C_CD", "phi_CD",
               "phi_C_CD", "cosTheta1", "cosTheta2", "phi1", "phi2"]


def cg_coef(j1, j2, m1, m2, j, m):
    if m1 + m2 != m or j < abs(j1 - j2) or j > j1 + j2:
        return 0.0
    if abs(m1) > j1 or abs(m2) > j2 or abs(m) > j:
        return 0.0
    pref = _msqrt((2 * j + 1) * _f(j1 + j2 - j) * _f(j + j1 - j2) * _f(j + j2 - j1) / _f(j1 + j2 + j + 1))
    pref *= _msqrt(_f(j + m) * _f(j - m) * _f(j1 - m1) * _f(j1 + m1) * _f(j2 - m2) * _f(j2 + m2))
    s = 0.0
    for k in range(max(0, j2 - j - m1, j1 + m2 - j), min(j1 + j2 - j, j1 - m1, j2 + m2) + 1):
        s += (-1) ** k / (_f(k) * _f(j1 + j2 - j - k) * _f(j1 - m1 - k) * _f(j2 + m2 - k) * _f(j - j2 + m1 + k) * _f(j - j1 - m2 + k))
    return pref * s


def ls_list(ja, jb, jc, pa, pb, pc):
    dl = 0 if pa * pb * pc == 1 else 1
    out = []
    for s in range(abs(jb - jc), jb + jc + 1):
        for l in range(abs(ja - s), ja + s + 1):
            if l % 2 == dl:
                out.append((l, s))
    return out


def _build_layout():
    layout, off = [], 0
    for (_, chain, m0, g0, J, Par) in RES:
        jc, jd, je = (0, 1, 1) if chain < 0 else ((1, 1, 0) if chain < 100 else (1, 0, 1))
        ls0 = ls_list(JA, J, jc, PA, Par, -1)
        ls1 = ls_list(J, jd, je, Par, -1, -1)
        layout.append(((off, ls0), (off + len(ls0), ls1)))
        off += len(ls0) + len(ls1)
    return layout, off


LAYOUT, NCOEF = _build_layout()


def H_ls_val(params, entry, ja, jb, jc, lb, lc):
    off, ls = entry
    tot = 0.0 + 0.0j
    nz = False
    for idx, (l, s) in enumerate(ls):
        c = cg_coef(jb, jc, lb, -lc, s, lb - lc) * cg_coef(l, s, 0, lb - lc, ja, lb - lc)
        if c != 0.0:
            nz = True
            tot = tot + c * (params[off + idx, 0] + 1j * params[off + idx, 1])
    return tot if nz else None


def wigner_d_val(j, m1, m2, c):
    if abs(m1) > j or abs(m2) > j:
        return None
    ch = np.sqrt((1.0 + c) * 0.5)
    sh = np.sqrt((1.0 - c) * 0.5)
    pref = _msqrt(_f(j + m1) * _f(j - m1) * _f(j + m2) * _f(j - m2))
    out = 0.0
    for s in range(max(0, m2 - m1), min(j - m1, j + m2) + 1):
        coef = pref * (-1) ** (m1 - m2 + s) / (_f(j + m2 - s) * _f(s) * _f(m1 - m2 + s) * _f(j - m1 - s))
        out = out + coef * ch ** (2 * j + m2 - m1 - 2 * s) * sh ** (m1 - m2 + 2 * s)
    return out


def getp_const(M0, M1, M2):
    s, d = M1 + M2, M1 - M2
    p = (M0 - s) * (M0 + s) * (M0 - d) * (M0 + d)
    return _msqrt(p) / (2.0 * M0)


def fit_d2_poly(lBC, lB2):
    """d2_{lBC,lB2}(c) = a0 + a1*c + a2*c^2 + b0*s + b1*s*c (exact for j=2)."""
    cs = np.linspace(-0.95, 0.95, 21)
    ss = np.sqrt(1 - cs**2)
    A = np.stack([np.ones_like(cs), cs, cs**2, ss, ss * cs], 1)
    y = np.array([wigner_d_val(2, lBC, lB2, c) for c in cs], float)
    coef, *_ = np.linalg.lstsq(A, y, rcond=None)
    assert np.abs(A @ coef - y).max() < 1e-10, (lBC, lB2)
    coef[np.abs(coef) < 1e-12] = 0.0
    return coef


# ---------------------------------------------------------------- expr DAG
MULT, ADD, SUB, MAX = "mult", "add", "subtract", "max"


class Prog:
    def __init__(self):
        self.nodes = []
        self.cache = {}

    def _emit(self, key):
        if key in self.cache:
            return self.cache[key]
        idx = len(self.nodes)
        self.nodes.append(key)
        self.cache[key] = idx
        return idx

    def inp(self, name):
        return self._emit(("IN", name))

    def tt(self, a, b, alu=MULT):
        if alu in (MULT, ADD):
            a, b = min(a, b), max(a, b)
        return self._emit(("TT", a, b, alu))

    def ts(self, a, s1, op0=MULT, s2=None, op1=None):
        s1 = float(s1)
        if s2 is not None:
            s2 = float(s2)
        if op0 == MULT and s1 == 1.0 and s2 is None:
            return a
        return self._emit(("TS", a, s1, op0, s2, op1))

    def stt(self, a, s, b, op0=MULT, op1=ADD):
        s = float(s)
        if op0 == MULT and s == 1.0:
            return self.tt(a, b, op1)
        return self._emit(("STT", a, s, b, op0, op1))

    def act(self, func, a, scale=1.0, bias=0.0):
        return self._emit(("ACT", func, a, float(scale), float(bias)))

    def rec(self, a):
        return self._emit(("REC", a))

    def deps(self, i):
        nd = self.nodes[i]
        op = nd[0]
        if op == "TT":
            return [nd[1], nd[2]]
        if op == "TS":
            return [nd[1]]
        if op == "STT":
            return [nd[1], nd[3]]
        if op == "ACT":
            return [nd[2]]
        if op == "REC":
            return [nd[1]]
        return []

    def evaluate(self, inputs, outputs, dtype=np.float64):
        vals = [None] * len(self.nodes)
        for i, nd in enumerate(self.nodes):
            op = nd[0]
            if op == "IN":
                vals[i] = np.asarray(inputs[nd[1]], dtype)
            elif op == "TT":
                _, a, b, alu = nd
                x, y = vals[a], vals[b]
                vals[i] = {MULT: lambda: x * y, ADD: lambda: x + y,
                           SUB: lambda: x - y,
                           MAX: lambda: np.maximum(x, y)}[alu]().astype(dtype)
            elif op == "TS":
                _, a, s1, op0, s2, op1 = nd
                x = vals[a]
                f = {MULT: lambda v, s: v * s, ADD: lambda v, s: v + s,
                     SUB: lambda v, s: v - s}
                r = f[op0](x, dtype(s1))
                if s2 is not None:
                    r = f[op1](r, dtype(s2))
                vals[i] = r.astype(dtype)
            elif op == "STT":
                _, a, s, b, op0, op1 = nd
                x, y = vals[a], vals[b]
                r = {MULT: lambda: x * dtype(s), ADD: lambda: x + dtype(s)}[op0]()
                r = {ADD: lambda: r + y, SUB: lambda: r - y,
                     MULT: lambda: r * y}[op1]()
                vals[i] = r.astype(dtype)
            elif op == "ACT":
                _, func, a, scale, bias = nd
                x = vals[a] * dtype(scale) + dtype(bias)
                if func == "Sin":
                    vals[i] = np.sin(x).astype(dtype)
                elif func == "Sqrt":
                    vals[i] = np.sqrt(np.maximum(x, 0)).astype(dtype)
                else:
                    raise ValueError(func)
            elif op == "REC":
                vals[i] = (dtype(1.0) / vals[nd[1]]).astype(dtype)
        return [vals[o] for o in outputs]

    def live_set(self, outputs):
        live = set()
        stack = list(outputs)
        while stack:
            i = stack.pop()
            if i in live:
                continue
            live.add(i)
            stack += self.deps(i)
        return live


# ---------------------------------------------------------------- program
def build_program(params):
    p = Prog()
    P = np.asarray(params, np.float64)
    H = H_ls_val
    L = LAYOUT
    H1_0 = {l: H(P, L[0][0], 1, 1, 0, l, 0) for l in (-1, 0, 1)}
    H2_0 = {(lD, lB): H(P, L[0][1], 1, 1, 1, lD, lB) for lD in (-1, 0, 1) for lB in (-1, 0, 1)}
    H1_1 = {(lBC, lD): H(P, L[1][0], 1, 2, 1, lBC, lD) for lBC in range(-2, 3) for lD in (-1, 0, 1)}
    H2_1 = {l: H(P, L[1][1], 2, 1, 0, l, 0) for l in (-1, 0, 1)}
    H1_2 = {(lCD, lB): H(P, L[2][0], 1, 1, 1, lCD, lB) for lCD in (-1, 0, 1) for lB in (-1, 0, 1)}
    H2_2 = {l: H(P, L[2][1], 1, 0, 1, 0, l) for l in (-1, 0, 1)}
    z = lambda x: 0j if x is None else x

    IN = {k: p.inp(k) for k in INPUT_NAMES}

    def bw_nodes(m_idx, m0, g0, M1, M2):
        s0, d0 = M1 + M2, M1 - M2
        q0 = getp_const(m0, M1, M2)
        K = g0 * m0 * m0 / (2.0 * q0)
        m2 = p.tt(m_idx, m_idx)
        t = p.ts(m2, -s0 * s0, ADD)
        u = p.ts(m2, -d0 * d0, ADD)
        pp = p.tt(t, u)
        g = p.act("Sqrt", pp, scale=K * K)
        dre = p.ts(m2, -1.0, MULT, m0 * m0, ADD)
        e = p.tt(m2, dre)
        e2 = p.tt(e, e)
        g2 = p.tt(g, g)
        dab = p.tt(e2, g2, ADD)
        dinv = p.rec(dab)
        w = p.tt(g, dinv)
        return (p.tt(w, e), p.tt(w, g))

    bw0 = bw_nodes(IN["m_BD"], 4.026, 0.025, M0_B, M0_D)
    bw1 = bw_nodes(IN["m_BC"], 2.4607, 0.0475, M0_B, M0_C)
    bw2 = bw_nodes(IN["m_CD"], 2.4232, 0.025, M0_C, M0_D)

    def sine(c_idx):
        c2 = p.tt(c_idx, c_idx)
        return p.act("Sqrt", c2, scale=-1.0, bias=1.0), c2

    cBD = IN["cosTheta_BD"];   sBD, _ = sine(cBD)
    cDD = IN["cosTheta_D_BD"]; sDD, _ = sine(cDD)
    cBCa = IN["cosTheta_BC"];  sBCa, _ = sine(cBCa)
    cBB = IN["cosTheta_B_BC"]; sBB, c2BB = sine(cBB)
    cCDa = IN["cosTheta_CD"];  sCDa, _ = sine(cCDa)
    cCC = IN["cosTheta_C_CD"]; sCC, _ = sine(cCC)
    ct1 = IN["cosTheta1"];     st1, _ = sine(ct1)
    ct2 = IN["cosTheta2"];     st2, _ = sine(ct2)

    # phases: sin(x) direct; cos(x) = Sin(pi/2 - |x|) (range-safe);
    # phi sums via addition formulas.
    def sincos(x_idx):
        s = p.act("Sin", x_idx)
        neg = p.ts(x_idx, -1.0)
        ax = p.tt(x_idx, neg, MAX)
        c = p.act("Sin", ax, scale=-1.0, bias=HALF_PI)
        return s, c

    sD_, cD_ = sincos(IN["phi_D_BD"])
    sB1, cB1 = sincos(IN["phi_BC"])
    sC1, cC1 = sincos(IN["phi_CD"])
    sp1, cp1 = sincos(IN["phi1"])
    sbb, cbb = sincos(IN["phi_B_BC"])
    sp2, cp2 = sincos(IN["phi2"])
    scc, ccc = sincos(IN["phi_C_CD"])
    sA = p.tt(p.tt(sp1, cbb), p.tt(cp1, sbb), ADD)
    cA = p.tt(p.tt(cp1, cbb), p.tt(sp1, sbb), SUB)
    sB_ = p.tt(p.tt(sp2, ccc), p.tt(cp2, scc), ADD)
    cB_ = p.tt(p.tt(cp2, ccc), p.tt(sp2, scc), SUB)
    s2q = p.tt(sB1, sB1)
    scq = p.tt(sB1, cB1)

    def d1_vals(c_idx, s_idx):
        return {"u": p.ts(c_idx, 0.5, MULT, 0.5, ADD),
                "v": p.ts(c_idx, -0.5, MULT, 0.5, ADD),
                "w": p.ts(s_idx, 1.0 / SQ2),
                "c": c_idx}

    vBD = d1_vals(cBD, sBD)
    vDD = d1_vals(cDD, sDD)
    vBC = d1_vals(cBCa, sBCa)
    vT1 = d1_vals(ct1, st1)
    vT2 = d1_vals(ct2, st2)
    vCD = d1_vals(cCDa, sCDa)
    vCC = d1_vals(cCC, sCC)

    prodcache = {}

    def vprod(va, ka, vb, kb, tag):
        key = (tag, ka, kb)
        if key not in prodcache:
            prodcache[key] = p.tt(va[ka], vb[kb])
        return prodcache[key]

    d1row = {1: {1: ("u", 1.0), 0: ("w", -1.0), -1: ("v", 1.0)},
             0: {1: ("w", 1.0), 0: ("c", 1.0), -1: ("w", -1.0)},
             -1: {1: ("v", 1.0), 0: ("w", 1.0), -1: ("u", 1.0)}}

    # ---- res0 ----
    b0r, b0i = bw0
    q1 = p.tt(b0r, cD_); q2 = p.tt(b0i, sD_); q3 = p.tt(b0i, cD_); q4 = p.tt(b0r, sD_)
    Bc = {0: (b0r, b0i),
          1: (p.tt(q1, q2, ADD), p.tt(q3, q4, SUB)),
          -1: (p.tt(q1, q2, SUB), p.tt(q3, q4, ADD))}

    amps = {}
    for lA in (-1, 1):
        for lB in (-1, 0, 1):
            for lD in (-1, 0, 1):
                mu = lD - lB
                if abs(mu) > 1 or z(H2_0[(lD, lB)]) == 0j:
                    continue
                terms = []
                for lDB in (-1, 0, 1):
                    h = z(H1_0[lDB]) * z(H2_0[(lD, lB)])
                    if h == 0j:
                        continue
                    ka, sa = d1row[lA][lDB]
                    kb, sb = d1row[lDB][mu]
                    terms.append((h * sa * sb, vprod(vBD, ka, vDD, kb, "r0")))
                if not terms:
                    continue
                gre = gim = None
                for (cf, prod) in terms:
                    cr, ci = float(cf.real), float(cf.imag)
                    gre = p.ts(prod, cr) if gre is None else p.stt(prod, cr, gre)
                    gim = p.ts(prod, ci) if gim is None else p.stt(prod, ci, gim)
                Br, Bi = Bc[lD]
                rr = p.tt(Br, gre); ii = p.tt(Bi, gim)
                ri = p.tt(Br, gim); ir = p.tt(Bi, gre)
                amps[(lA, lB, lD)] = (p.tt(rr, ii, SUB), p.tt(ri, ir, ADD))

    # ---- res1 ----
    b1r, b1i = bw1
    r1 = p.tt(b1r, cA); r2 = p.tt(b1i, sA); r3 = p.tt(b1i, cA); r4 = p.tt(b1r, sA)
    P1 = {1: (p.tt(r1, r2, ADD), p.tt(r3, r4, SUB)),
          -1: (p.tt(r1, r2, SUB), p.tt(r3, r4, ADD))}

    cU = {}
    for lBC in (-2, -1, 1, 2):
        for lD in (-1, 0, 1):
            if abs(lBC - lD) > 1:
                continue
            h = z(H1_1[(lBC, lD)])
            if h == 0j:
                continue
            hr, hi = float(h.real), float(h.imag)
            if lBC == 1:
                re = p.stt(sB1, hi, p.ts(cB1, hr))
                im = p.stt(sB1, -hr, p.ts(cB1, hi))
            elif lBC == -1:
                re = p.stt(sB1, -hi, p.ts(cB1, hr))
                im = p.stt(sB1, hr, p.ts(cB1, hi))
            else:
                sg = 1.0 if lBC == 2 else -1.0
                re = p.stt(scq, 2 * hi * sg, p.ts(s2q, -2 * hr, MULT, hr, ADD))
                im = p.stt(scq, -2 * hr * sg, p.ts(s2q, -2 * hi, MULT, hi, ADD))
            cU[(lBC, lD)] = (re, im)

    scB = p.tt(sBB, cBB)
    d2cache = {}

    def d2val(lBC, lB2):
        key = (lBC, lB2)
        if key in d2cache:
            return d2cache[key]
        mkey = (-lBC, -lB2)
        if mkey in d2cache:
            base, sgn = d2cache[mkey]
            d2cache[key] = (base, sgn * (-1.0) ** (lBC - lB2))
            return d2cache[key]
        a0, a1, a2, b0, b1 = fit_d2_poly(lBC, lB2)
        node = None
        for (cf, src) in ((a1, cBB), (a2, c2BB), (b0, sBB), (b1, scB)):
            if cf == 0.0:
                continue
            node = p.ts(src, cf) if node is None else p.stt(src, cf, node)
        if a0 != 0.0:
            node = p.ts(node, 1.0, MULT, a0, ADD)
        d2cache[key] = (node, 1.0)
        return d2cache[key]

    W = {}
    for lA in (-1, 1):
        for lD in (-1, 0, 1):
            for lB2 in (-1, 1):
                h2 = z(H2_1[lB2])
                if h2 == 0j:
                    continue
                h2r, h2i = float(h2.real), float(h2.imag)
                ev_terms, const_terms = [], []
                for lBC in range(max(-2, lD - 1), min(2, lD + 1) + 1):
                    nu = lBC - lD
                    ka, sa = d1row[lA][nu]
                    d2n, sgn2 = d2val(lBC, lB2)
                    ddp = p.tt(vBC[ka], d2n)
                    sc = sa * sgn2
                    if lBC == 0:
                        h = z(H1_1[(lBC, lD)]) * h2
                        if h != 0j:
                            const_terms.append((h * sc, ddp))
                    elif (lBC, lD) in cU:
                        ev_terms.append((sc, cU[(lBC, lD)], ddp))

                def wsum(comp):
                    node = None
                    for (sc, (ur, ui), ddp) in ev_terms:
                        tr = p.tt(ur, ddp)
                        ti = p.tt(ui, ddp)
                        cfr = sc * (h2r if comp == "re" else h2i)
                        cfi = sc * (-h2i if comp == "re" else h2r)
                        if cfr != 0.0:
                            node = p.ts(tr, cfr) if node is None else p.stt(tr, cfr, node)
                        if cfi != 0.0:
                            node = p.ts(ti, cfi) if node is None else p.stt(ti, cfi, node)
                    for (cf0, ddp) in const_terms:
                        cf = float(cf0.real) if comp == "re" else float(cf0.imag)
                        if cf == 0.0:
                            continue
                        node = p.ts(ddp, cf) if node is None else p.stt(ddp, cf, node)
                    return node
                wre, wim = wsum("re"), wsum("im")
                if wre is None and wim is None:
                    continue
                W[(lA, lD, lB2)] = (wre, wim)

    Zc = {}
    for key, (wre, wim) in W.items():
        lA, lD, lB2 = key
        Pr, Pi = P1[lB2]
        rr = p.tt(Pr, wre); ii = p.tt(Pi, wim)
        ri = p.tt(Pr, wim); ir = p.tt(Pi, wre)
        Zc[key] = (p.tt(rr, ii, SUB), p.tt(ri, ir, ADD))

    for lA in (-1, 1):
        for lB in (-1, 0, 1):
            for lD in (-1, 0, 1):
                acc = amps.get((lA, lB, lD))
                for lB2 in (-1, 1):
                    if (lA, lD, lB2) not in Zc:
                        continue
                    zr, zi = Zc[(lA, lD, lB2)]
                    ka, sa = d1row[lB2][lB]
                    val = vT1[ka]
                    tre = p.tt(val, zr); tim = p.tt(val, zi)
                    if acc is None:
                        acc = ((p.ts(tre, sa), p.ts(tim, sa)) if sa != 1.0 else (tre, tim))
                    else:
                        op = ADD if sa > 0 else SUB
                        acc = (p.tt(acc[0], tre, op), p.tt(acc[1], tim, op))
                amps[(lA, lB, lD)] = acc

    # ---- res2 ----
    b2r, b2i = bw2
    u1 = p.tt(b2r, cB_); u2 = p.tt(b2i, sB_); u3 = p.tt(b2i, cB_); u4 = p.tt(b2r, sB_)
    P2 = {1: (p.tt(u1, u2, ADD), p.tt(u3, u4, SUB)),
          -1: (p.tt(u1, u2, SUB), p.tt(u3, u4, ADD)),
          0: bw2}

    cU2 = {}
    for lCD in (-1, 1):
        for lB in (-1, 0, 1):
            if abs(lCD - lB) > 1:
                continue
            h = z(H1_2[(lCD, lB)])
            if h == 0j:
                continue
            hr, hi = float(h.real), float(h.imag)
            if lCD == 1:
                re = p.stt(sC1, hi, p.ts(cC1, hr))
                im = p.stt(sC1, -hr, p.ts(cC1, hi))
            else:
                re = p.stt(sC1, -hi, p.ts(cC1, hr))
                im = p.stt(sC1, hr, p.ts(cC1, hi))
            cU2[(lCD, lB)] = (re, im)

    V = {}
    for lA in (-1, 1):
        for lB in (-1, 0, 1):
            for lC2 in (-1, 0, 1):
                h2 = z(H2_2[lC2])
                if h2 == 0j:
                    continue
                h2r, h2i = float(h2.real), float(h2.imag)
                ev_terms, const_terms = [], []
                for lCD in (-1, 0, 1):
                    if abs(lCD - lB) > 1:
                        continue
                    h1 = z(H1_2[(lCD, lB)])
                    if h1 is None or h1 == 0j:
                        continue
                    nu = lCD - lB
                    ka, sa = d1row[lA][nu]
                    kb, sb = d1row[lCD][lC2]
                    ddp = vprod(vCD, ka, vCC, kb, "r2")
                    if lCD == 0:
                        const_terms.append((h1 * h2 * sa * sb, ddp))
                    else:
                        ev_terms.append((sa * sb, cU2[(lCD, lB)], ddp))

                def vsum(comp):
                    node = None
                    for (sc, (ur, ui), ddp) in ev_terms:
                        tr = p.tt(ur, ddp)
                        ti = p.tt(ui, ddp)
                        cfr = sc * (h2r if comp == "re" else h2i)
                        cfi = sc * (-h2i if comp == "re" else h2r)
                        if cfr != 0.0:
                            node = p.ts(tr, cfr) if node is None else p.stt(tr, cfr, node)
                        if cfi != 0.0:
                            node = p.ts(ti, cfi) if node is None else p.stt(ti, cfi, node)
                    for (cf0, ddp) in const_terms:
                        cf = float(cf0.real) if comp == "re" else float(cf0.imag)
                        if cf == 0.0:
                            continue
                        node = p.ts(ddp, cf) if node is None else p.stt(ddp, cf, node)
                    return node
                vre, vim = vsum("re"), vsum("im")
                if vre is None and vim is None:
                    continue
                V[(lA, lB, lC2)] = (vre, vim)

    Z2 = {}
    for key, (vre, vim) in V.items():
        lA, lB, lC2 = key
        Pr, Pi = P2[lC2]
        rr = p.tt(Pr, vre); ii = p.tt(Pi, vim)
        ri = p.tt(Pr, vim); ir = p.tt(Pi, vre)
        Z2[key] = (p.tt(rr, ii, SUB), p.tt(ri, ir, ADD))

    for lA in (-1, 1):
        for lB in (-1, 0, 1):
            for lD in (-1, 0, 1):
                acc = amps.get((lA, lB, lD))
                for lC2 in (-1, 0, 1):
                    if (lA, lB, lC2) not in Z2:
                        continue
                    zr, zi = Z2[(lA, lB, lC2)]
                    ka, sa = d1row[lC2][lD]
                    val = vT2[ka]
                    tre = p.tt(val, zr); tim = p.tt(val, zi)
                    if acc is None:
                        acc = ((p.ts(tre, sa), p.ts(tim, sa)) if sa != 1.0 else (tre, tim))
                    else:
                        op = ADD if sa > 0 else SUB
                        acc = (p.tt(acc[0], tre, op), p.tt(acc[1], tim, op))
                amps[(lA, lB, lD)] = acc

    total = None
    for key in sorted(amps):
        ar, ai = amps[key]
        s1 = p.tt(ar, ar)
        total = s1 if total is None else p.tt(total, s1, ADD)
        s2 = p.tt(ai, ai)
        total = p.tt(total, s2, ADD)
    return p, total


# ---------------------------------------------------------------- codegen
def emit_bass(params):
    import concourse.bass as bass
    import concourse.tile as tile
    from concourse import bacc, mybir

    prog, out_idx = build_program(params)
    live = prog.live_set([out_idx])
    order = [i for i in range(len(prog.nodes)) if i in live]
    last_use = {}
    for i in order:
        for d in prog.deps(i):
            last_use[d] = i
    last_use[out_idx] = len(prog.nodes) + 1

    # max live for pool sizing
    alive, maxlive = set(), 0
    for i in order:
        alive.add(i)
        for d in prog.deps(i):
            if d in alive and last_use.get(d) == i:
                alive.discard(d)
        maxlive = max(maxlive, len(alive))
    nbufs = maxlive + 10
    assert nbufs * 512 * 4 <= 222 * 1024, nbufs

    nc = bacc.Bacc("TRN2", target_bir_lowering=False, debug=False,
                   num_devices=N_CORES)
    in_aps = {k: nc.dram_tensor(k, [128, COLS], mybir.dt.float32,
                                kind="ExternalInput").ap() for k in INPUT_NAMES}
    out_ap = nc.dram_tensor("total", [128, COLS], mybir.dt.float32,
                            kind="ExternalOutput").ap()

    # const APs for activation biases (pi/2 for cos; every TS add/bias we
    # route to the scalar engine needs its own const tile)
    bias_vals = {HALF_PI}
    for i in order:
        nd = prog.nodes[i]
        if nd[0] == "TS":
            _, a, s1, op0, s2, op1 = nd
            if op0 == ADD and s2 is None:
                bias_vals.add(float(s1))
            elif op0 == MULT and s2 is not None and op1 == ADD:
                bias_vals.add(float(s2))
    for value in sorted(bias_vals):
        key = (mybir.dt.float32, value)
        if key not in nc.const_aps.aps:
            t = nc.alloc_sbuf_tensor(f"constb-{len(nc.const_aps.aps)}", [128, 1], mybir.dt.float32)
            nc.gpsimd.memset(t.ap(), value)
            nc.const_aps.aps[key] = t.ap()
    nc.all_engine_barrier()

    ALU = {MULT: mybir.AluOpType.mult, ADD: mybir.AluOpType.add,
           SUB: mybir.AluOpType.subtract, MAX: mybir.AluOpType.max}
    AFUNC = {"Sin": mybir.ActivationFunctionType.Sin,
             "Sqrt": mybir.ActivationFunctionType.Sqrt}

    with tile.TileContext(nc) as tc:
        with ExitStack() as ctx:
            pool = ctx.enter_context(tc.tile_pool(name="slab", bufs=nbufs))
            for (c0, fd) in COL_TILES:
                tiles = {}
                for i in order:
                    nd = prog.nodes[i]
                    op = nd[0]
                    t = pool.tile([128, fd], mybir.dt.float32, tag="slab")
                    if op == "IN":
                        nc.sync.dma_start(t[:], in_aps[nd[1]][:, c0:c0 + fd])
                    elif op == "TT":
                        if nd[1] == nd[2] and nd[3] == MULT:
                            nc.scalar.activation(t[:], tiles[nd[1]][:],
                                                 mybir.ActivationFunctionType.Square,
                                                 bias=0.0, scale=1.0)
                        else:
                            nc.vector.tensor_tensor(t[:], tiles[nd[1]][:], tiles[nd[2]][:], ALU[nd[3]])
                    elif op == "TS":
                        _, a, s1, op0, s2, op1 = nd
                        # scalar-engine offload: the vector engine is the
                        # bottleneck and ACT sits idle.
                        if s2 is None and op0 == MULT:
                            nc.scalar.activation(t[:], tiles[a][:],
                                                 mybir.ActivationFunctionType.Copy,
                                                 bias=0.0, scale=s1)
                        elif s2 is None and op0 == ADD:
                            nc.scalar.activation(t[:], tiles[a][:],
                                                 mybir.ActivationFunctionType.Identity,
                                                 bias=s1, scale=1.0)
                        elif op0 == MULT and op1 == ADD:
                            nc.scalar.activation(t[:], tiles[a][:],
                                                 mybir.ActivationFunctionType.Identity,
                                                 bias=s2, scale=s1)
                        else:
                            nc.vector.tensor_scalar(t[:], tiles[a][:], s1, s2, ALU[op0], ALU[op1])
                    elif op == "STT":
                        _, a, s, b, op0, op1 = nd
                        nc.vector.scalar_tensor_tensor(t[:], tiles[a][:], s, tiles[b][:], ALU[op0], ALU[op1])
                    elif op == "ACT":
                        _, func, a, scale, bias = nd
                        nc.scalar.activation(t[:], tiles[a][:], AFUNC[func], bias=bias, scale=scale)
                    elif op == "REC":
                        nc.vector.reciprocal(t[:], tiles[nd[1]][:])
                    tiles[i] = t
                    if i == out_idx:
                        nc.sync.dma_start(out_ap[:, c0:c0 + fd], t[:])
    nc.compile()
    return nc


# ---------------------------------------------------------------- kernel
def kernel(**inputs):
    params = np.asarray(inputs["params"], np.float32)
    nc = emit_bass(params)

    from concourse.bass_utils import run_bass_kernel_spmd

    in_maps = []
    for core in range(N_CORES):
        m = {}
        for k in INPUT_NAMES:
            arr = np.asarray(inputs[k], np.float32)[core * PER_CORE:(core + 1) * PER_CORE]
            pad = np.empty(PER_CORE_PAD, np.float32)
            pad[:PER_CORE] = arr
            pad[PER_CORE:] = arr[0]
            m[k] = pad.reshape(128, COLS)
        in_maps.append(m)

    trace = bool(os.environ.get("KERNEL_TRACE"))
    if trace:
        try:
            import sys, types
            from trn_agent_boot.trn_boot import _ntff_profile_via_ctypes
            hook = _ntff_profile_via_ctypes('/opt/axon/libaxon_pjrt.so')
            mmod = types.ModuleType('antenv.axon_hooks')
            mmod.get_axon_ntff_profile_hook = lambda: hook
            mmod.set_axon_ntff_profile_hook = lambda h: None
            sys.modules.setdefault('antenv.axon_hooks', mmod)
        except Exception as e:
            print("profile shim failed:", e)
            trace = False

    res = run_bass_kernel_spmd(nc, in_maps, core_ids=list(range(N_CORES)),
                               trace=trace)
    if trace and res.exec_time_ns is not None:
        print(f"HW exec time: {res.exec_time_ns} ns")

    out = np.empty(N_FULL, np.float32)
    for core in range(N_CORES):
        out[core * PER_CORE:(core + 1) * PER_CORE] = \
            res.results[core]["total"].reshape(-1)[:PER_CORE]
    return out


# revision 4
# speedup vs baseline: 1.4195x; 1.1070x over previous
"""Trainium2 Bass kernel for nn_AllAmplitude: per-event |amplitude|^2 sum.

Strategy: all params-derived complex couplings are computed on the host and
baked into the compiled kernel as immediate constants. The per-event math is
a straight-line elementwise program (DVE + ACT engines), evaluated on
[128 x FD] tiles; events are sharded across 8 NeuronCores.
"""
import os
import numpy as np
from math import factorial as _f, sqrt as _msqrt
from contextlib import ExitStack

# ---------------------------------------------------------------- constants
JA, JB, JC, JD = 1, 1, 0, 1
PA, PB, PC, PD = -1, -1, -1, -1
M0_B, M0_C, M0_D = 2.01026, 0.13957061, 2.00685
RES = [("Zc_4025", -1, 4.026, 0.025, 1, 1),
       ("D2_2460", 21, 2.4607, 0.0475, 2, 1),
       ("D1_2420", 121, 2.4232, 0.025, 1, 1)]
N_FULL = 2000000
N_CORES = 8
PER_CORE = N_FULL // N_CORES            # 250000
COLS = 1956                             # 128*1956 = 250368 >= 250000
PER_CORE_PAD = 128 * COLS
COL_TILES = [(0, 512), (512, 512), (1024, 512), (1536, 420)]
SQ2 = float(np.sqrt(2.0))
HALF_PI = float(np.pi / 2)

INPUT_NAMES = ["m_BC", "m_BD", "m_CD", "cosTheta_BC", "cosTheta_B_BC",
               "phi_BC", "phi_B_BC", "cosTheta_BD", "cosTheta_D_BD",
               "phi_D_BD", "cosTheta_CD", "cosTheta_C_CD", "phi_CD",
               "phi_C_CD", "cosTheta1", "cosTheta2", "phi1", "phi2"]


def cg_coef(j1, j2, m1, m2, j, m):
    if m1 + m2 != m or j < abs(j1 - j2) or j > j1 + j2:
        return 0.0
    if abs(m1) > j1 or abs(m2) > j2 or abs(m) > j:
        return 0.0
    pref = _msqrt((2 * j + 1) * _f(j1 + j2 - j) * _f(j + j1 - j2) * _f(j + j2 - j1) / _f(j1 + j2 + j + 1))
    pref *= _msqrt(_f(j + m) * _f(j - m) * _f(j1 - m1) * _f(j1 + m1) * _f(j2 - m2) * _f(j2 + m2))
    s = 0.0
    for k in range(max(0, j2 - j - m1, j1 + m2 - j), min(j1 + j2 - j, j1 - m1, j2 + m2) + 1):
        s += (-1) ** k / (_f(k) * _f(j1 + j2 - j - k) * _f(j1 - m1 - k) * _f(j2 + m2 - k) * _f(j - j2 + m1 + k) * _f(j - j1 - m2 + k))
    return pref * s


def ls_list(ja, jb, jc, pa, pb, pc):
    dl = 0 if pa * pb * pc == 1 else 1
    out = []
    for s in range(abs(jb - jc), jb + jc + 1):
        for l in range(abs(ja - s), ja + s + 1):
            if l % 2 == dl:
                out.append((l, s))
    return out


def _build_layout():
    layout, off = [], 0
    for (_, chain, m0, g0, J, Par) in RES:
        jc, jd, je = (0, 1, 1) if chain < 0 else ((1, 1, 0) if chain < 100 else (1, 0, 1))
        ls0 = ls_list(JA, J, jc, PA, Par, -1)
        ls1 = ls_list(J, jd, je, Par, -1, -1)
        layout.append(((off, ls0), (off + len(ls0), ls1)))
        off += len(ls0) + len(ls1)
    return layout, off


LAYOUT, NCOEF = _build_layout()


def H_ls_val(params, entry, ja, jb, jc, lb, lc):
    off, ls = entry
    tot = 0.0 + 0.0j
    nz = False
    for idx, (l, s) in enumerate(ls):
        c = cg_coef(jb, jc, lb, -lc, s, lb - lc) * cg_coef(l, s, 0, lb - lc, ja, lb - lc)
        if c != 0.0:
            nz = True
            tot = tot + c * (params[off + idx, 0] + 1j * params[off + idx, 1])
    return tot if nz else None


def wigner_d_val(j, m1, m2, c):
    if abs(m1) > j or abs(m2) > j:
        return None
    ch = np.sqrt((1.0 + c) * 0.5)
    sh = np.sqrt((1.0 - c) * 0.5)
    pref = _msqrt(_f(j + m1) * _f(j - m1) * _f(j + m2) * _f(j - m2))
    out = 0.0
    for s in range(max(0, m2 - m1), min(j - m1, j + m2) + 1):
        coef = pref * (-1) ** (m1 - m2 + s) / (_f(j + m2 - s) * _f(s) * _f(m1 - m2 + s) * _f(j - m1 - s))
        out = out + coef * ch ** (2 * j + m2 - m1 - 2 * s) * sh ** (m1 - m2 + 2 * s)
    return out


def getp_const(M0, M1, M2):
    s, d = M1 + M2, M1 - M2
    p = (M0 - s) * (M0 + s) * (M0 - d) * (M0 + d)
    return _msqrt(p) / (2.0 * M0)


def fit_d2_poly(lBC, lB2):
    """d2_{lBC,lB2}(c) = a0 + a1*c + a2*c^2 + b0*s + b1*s*c (exact for j=2)."""
    cs = np.linspace(-0.95, 0.95, 21)
    ss = np.sqrt(1 - cs**2)
    A = np.stack([np.ones_like(cs), cs, cs**2, ss, ss * cs], 1)
    y = np.array([wigner_d_val(2, lBC, lB2, c) for c in cs], float)
    coef, *_ = np.linalg.lstsq(A, y, rcond=None)
    assert np.abs(A @ coef - y).max() < 1e-10, (lBC, lB2)
    coef[np.abs(coef) < 1e-12] = 0.0
    return coef


# ---------------------------------------------------------------- expr DAG
MULT, ADD, SUB, MAX = "mult", "add", "subtract", "max"


class Prog:
    def __init__(self):
        self.nodes = []
        self.cache = {}

    def _emit(self, key):
        if key in self.cache:
            return self.cache[key]
        idx = len(self.nodes)
        self.nodes.append(key)
        self.cache[key] = idx
        return idx

    def inp(self, name):
        return self._emit(("IN", name))

    def tt(self, a, b, alu=MULT):
        if alu in (MULT, ADD):
            a, b = min(a, b), max(a, b)
        return self._emit(("TT", a, b, alu))

    def ts(self, a, s1, op0=MULT, s2=None, op1=None):
        s1 = float(s1)
        if s2 is not None:
            s2 = float(s2)
        if op0 == MULT and s1 == 1.0 and s2 is None:
            return a
        return self._emit(("TS", a, s1, op0, s2, op1))

    def stt(self, a, s, b, op0=MULT, op1=ADD):
        s = float(s)
        if op0 == MULT and s == 1.0:
            return self.tt(a, b, op1)
        return self._emit(("STT", a, s, b, op0, op1))

    def act(self, func, a, scale=1.0, bias=0.0):
        return self._emit(("ACT", func, a, float(scale), float(bias)))

    def rec(self, a):
        return self._emit(("REC", a))

    def deps(self, i):
        nd = self.nodes[i]
        op = nd[0]
        if op == "TT":
            return [nd[1], nd[2]]
        if op == "TS":
            return [nd[1]]
        if op == "STT":
            return [nd[1], nd[3]]
        if op == "ACT":
            return [nd[2]]
        if op == "REC":
            return [nd[1]]
        return []

    def evaluate(self, inputs, outputs, dtype=np.float64):
        vals = [None] * len(self.nodes)
        for i, nd in enumerate(self.nodes):
            op = nd[0]
            if op == "IN":
                vals[i] = np.asarray(inputs[nd[1]], dtype)
            elif op == "TT":
                _, a, b, alu = nd
                x, y = vals[a], vals[b]
                vals[i] = {MULT: lambda: x * y, ADD: lambda: x + y,
                           SUB: lambda: x - y,
                           MAX: lambda: np.maximum(x, y)}[alu]().astype(dtype)
            elif op == "TS":
                _, a, s1, op0, s2, op1 = nd
                x = vals[a]
                f = {MULT: lambda v, s: v * s, ADD: lambda v, s: v + s,
                     SUB: lambda v, s: v - s}
                r = f[op0](x, dtype(s1))
                if s2 is not None:
                    r = f[op1](r, dtype(s2))
                vals[i] = r.astype(dtype)
            elif op == "STT":
                _, a, s, b, op0, op1 = nd
                x, y = vals[a], vals[b]
                r = {MULT: lambda: x * dtype(s), ADD: lambda: x + dtype(s)}[op0]()
                r = {ADD: lambda: r + y, SUB: lambda: r - y,
                     MULT: lambda: r * y}[op1]()
                vals[i] = r.astype(dtype)
            elif op == "ACT":
                _, func, a, scale, bias = nd
                x = vals[a] * dtype(scale) + dtype(bias)
                if func == "Sin":
                    vals[i] = np.sin(x).astype(dtype)
                elif func == "Sqrt":
                    vals[i] = np.sqrt(np.maximum(x, 0)).astype(dtype)
                else:
                    raise ValueError(func)
            elif op == "REC":
                vals[i] = (dtype(1.0) / vals[nd[1]]).astype(dtype)
        return [vals[o] for o in outputs]

    def live_set(self, outputs):
        live = set()
        stack = list(outputs)
        while stack:
            i = stack.pop()
            if i in live:
                continue
            live.add(i)
            stack += self.deps(i)
        return live


# ---------------------------------------------------------------- program
def build_program(params):
    p = Prog()
    P = np.asarray(params, np.float64)
    H = H_ls_val
    L = LAYOUT
    H1_0 = {l: H(P, L[0][0], 1, 1, 0, l, 0) for l in (-1, 0, 1)}
    H2_0 = {(lD, lB): H(P, L[0][1], 1, 1, 1, lD, lB) for lD in (-1, 0, 1) for lB in (-1, 0, 1)}
    H1_1 = {(lBC, lD): H(P, L[1][0], 1, 2, 1, lBC, lD) for lBC in range(-2, 3) for lD in (-1, 0, 1)}
    H2_1 = {l: H(P, L[1][1], 2, 1, 0, l, 0) for l in (-1, 0, 1)}
    H1_2 = {(lCD, lB): H(P, L[2][0], 1, 1, 1, lCD, lB) for lCD in (-1, 0, 1) for lB in (-1, 0, 1)}
    H2_2 = {l: H(P, L[2][1], 1, 0, 1, 0, l) for l in (-1, 0, 1)}
    z = lambda x: 0j if x is None else x

    IN = {k: p.inp(k) for k in INPUT_NAMES}

    def bw_nodes(m_idx, m0, g0, M1, M2):
        s0, d0 = M1 + M2, M1 - M2
        q0 = getp_const(m0, M1, M2)
        K = g0 * m0 * m0 / (2.0 * q0)
        m2 = p.tt(m_idx, m_idx)
        t = p.ts(m2, -s0 * s0, ADD)
        u = p.ts(m2, -d0 * d0, ADD)
        pp = p.tt(t, u)
        g = p.act("Sqrt", pp, scale=K * K)
        dre = p.ts(m2, -1.0, MULT, m0 * m0, ADD)
        e = p.tt(m2, dre)
        e2 = p.tt(e, e)
        g2 = p.tt(g, g)
        dab = p.tt(e2, g2, ADD)
        dinv = p.rec(dab)
        w = p.tt(g, dinv)
        return (p.tt(w, e), p.tt(w, g))

    bw0 = bw_nodes(IN["m_BD"], 4.026, 0.025, M0_B, M0_D)
    bw1 = bw_nodes(IN["m_BC"], 2.4607, 0.0475, M0_B, M0_C)
    bw2 = bw_nodes(IN["m_CD"], 2.4232, 0.025, M0_C, M0_D)

    def sine(c_idx):
        c2 = p.tt(c_idx, c_idx)
        return p.act("Sqrt", c2, scale=-1.0, bias=1.0), c2

    cBD = IN["cosTheta_BD"];   sBD, _ = sine(cBD)
    cDD = IN["cosTheta_D_BD"]; sDD, _ = sine(cDD)
    cBCa = IN["cosTheta_BC"];  sBCa, _ = sine(cBCa)
    cBB = IN["cosTheta_B_BC"]; sBB, c2BB = sine(cBB)
    cCDa = IN["cosTheta_CD"];  sCDa, _ = sine(cCDa)
    cCC = IN["cosTheta_C_CD"]; sCC, _ = sine(cCC)
    ct1 = IN["cosTheta1"];     st1, _ = sine(ct1)
    ct2 = IN["cosTheta2"];     st2, _ = sine(ct2)

    # phases: sin(x) direct; cos(x) = Sin(pi/2 - |x|) (range-safe);
    # phi sums via addition formulas.
    def sincos(x_idx):
        s = p.act("Sin", x_idx)
        neg = p.ts(x_idx, -1.0)
        ax = p.tt(x_idx, neg, MAX)
        c = p.act("Sin", ax, scale=-1.0, bias=HALF_PI)
        return s, c

    sD_, cD_ = sincos(IN["phi_D_BD"])
    sB1, cB1 = sincos(IN["phi_BC"])
    sC1, cC1 = sincos(IN["phi_CD"])
    sp1, cp1 = sincos(IN["phi1"])
    sbb, cbb = sincos(IN["phi_B_BC"])
    sp2, cp2 = sincos(IN["phi2"])
    scc, ccc = sincos(IN["phi_C_CD"])
    sA = p.tt(p.tt(sp1, cbb), p.tt(cp1, sbb), ADD)
    cA = p.tt(p.tt(cp1, cbb), p.tt(sp1, sbb), SUB)
    sB_ = p.tt(p.tt(sp2, ccc), p.tt(cp2, scc), ADD)
    cB_ = p.tt(p.tt(cp2, ccc), p.tt(sp2, scc), SUB)
    s2q = p.tt(sB1, sB1)
    scq = p.tt(sB1, cB1)

    def d1_vals(c_idx, s_idx):
        return {"u": p.ts(c_idx, 0.5, MULT, 0.5, ADD),
                "v": p.ts(c_idx, -0.5, MULT, 0.5, ADD),
                "w": p.ts(s_idx, 1.0 / SQ2),
                "c": c_idx}

    vBD = d1_vals(cBD, sBD)
    vDD = d1_vals(cDD, sDD)
    vBC = d1_vals(cBCa, sBCa)
    vT1 = d1_vals(ct1, st1)
    vT2 = d1_vals(ct2, st2)
    vCD = d1_vals(cCDa, sCDa)
    vCC = d1_vals(cCC, sCC)

    prodcache = {}

    def vprod(va, ka, vb, kb, tag):
        key = (tag, ka, kb)
        if key not in prodcache:
            prodcache[key] = p.tt(va[ka], vb[kb])
        return prodcache[key]

    d1row = {1: {1: ("u", 1.0), 0: ("w", -1.0), -1: ("v", 1.0)},
             0: {1: ("w", 1.0), 0: ("c", 1.0), -1: ("w", -1.0)},
             -1: {1: ("v", 1.0), 0: ("w", 1.0), -1: ("u", 1.0)}}

    # ---- res0 ----
    p.region_start = len(p.nodes)   # assembly stage begins: bf16-eligible
    b0r, b0i = bw0
    q1 = p.tt(b0r, cD_); q2 = p.tt(b0i, sD_); q3 = p.tt(b0i, cD_); q4 = p.tt(b0r, sD_)
    Bc = {0: (b0r, b0i),
          1: (p.tt(q1, q2, ADD), p.tt(q3, q4, SUB)),
          -1: (p.tt(q1, q2, SUB), p.tt(q3, q4, ADD))}

    amps = {}
    for lA in (-1, 1):
        for lB in (-1, 0, 1):
            for lD in (-1, 0, 1):
                mu = lD - lB
                if abs(mu) > 1 or z(H2_0[(lD, lB)]) == 0j:
                    continue
                terms = []
                for lDB in (-1, 0, 1):
                    h = z(H1_0[lDB]) * z(H2_0[(lD, lB)])
                    if h == 0j:
                        continue
                    ka, sa = d1row[lA][lDB]
                    kb, sb = d1row[lDB][mu]
                    terms.append((h * sa * sb, vprod(vBD, ka, vDD, kb, "r0")))
                if not terms:
                    continue
                gre = gim = None
                for (cf, prod) in terms:
                    cr, ci = float(cf.real), float(cf.imag)
                    gre = p.ts(prod, cr) if gre is None else p.stt(prod, cr, gre)
                    gim = p.ts(prod, ci) if gim is None else p.stt(prod, ci, gim)
                Br, Bi = Bc[lD]
                rr = p.tt(Br, gre); ii = p.tt(Bi, gim)
                ri = p.tt(Br, gim); ir = p.tt(Bi, gre)
                amps[(lA, lB, lD)] = (p.tt(rr, ii, SUB), p.tt(ri, ir, ADD))

    # ---- res1 ----
    b1r, b1i = bw1
    r1 = p.tt(b1r, cA); r2 = p.tt(b1i, sA); r3 = p.tt(b1i, cA); r4 = p.tt(b1r, sA)
    P1 = {1: (p.tt(r1, r2, ADD), p.tt(r3, r4, SUB)),
          -1: (p.tt(r1, r2, SUB), p.tt(r3, r4, ADD))}

    cU = {}
    for lBC in (-2, -1, 1, 2):
        for lD in (-1, 0, 1):
            if abs(lBC - lD) > 1:
                continue
            h = z(H1_1[(lBC, lD)])
            if h == 0j:
                continue
            hr, hi = float(h.real), float(h.imag)
            if lBC == 1:
                re = p.stt(sB1, hi, p.ts(cB1, hr))
                im = p.stt(sB1, -hr, p.ts(cB1, hi))
            elif lBC == -1:
                re = p.stt(sB1, -hi, p.ts(cB1, hr))
                im = p.stt(sB1, hr, p.ts(cB1, hi))
            else:
                sg = 1.0 if lBC == 2 else -1.0
                re = p.stt(scq, 2 * hi * sg, p.ts(s2q, -2 * hr, MULT, hr, ADD))
                im = p.stt(scq, -2 * hr * sg, p.ts(s2q, -2 * hi, MULT, hi, ADD))
            cU[(lBC, lD)] = (re, im)

    scB = p.tt(sBB, cBB)
    d2cache = {}

    def d2val(lBC, lB2):
        key = (lBC, lB2)
        if key in d2cache:
            return d2cache[key]
        mkey = (-lBC, -lB2)
        if mkey in d2cache:
            base, sgn = d2cache[mkey]
            d2cache[key] = (base, sgn * (-1.0) ** (lBC - lB2))
            return d2cache[key]
        a0, a1, a2, b0, b1 = fit_d2_poly(lBC, lB2)
        node = None
        for (cf, src) in ((a1, cBB), (a2, c2BB), (b0, sBB), (b1, scB)):
            if cf == 0.0:
                continue
            node = p.ts(src, cf) if node is None else p.stt(src, cf, node)
        if a0 != 0.0:
            node = p.ts(node, 1.0, MULT, a0, ADD)
        d2cache[key] = (node, 1.0)
        return d2cache[key]

    W = {}
    for lA in (-1, 1):
        for lD in (-1, 0, 1):
            for lB2 in (-1, 1):
                h2 = z(H2_1[lB2])
                if h2 == 0j:
                    continue
                h2r, h2i = float(h2.real), float(h2.imag)
                ev_terms, const_terms = [], []
                for lBC in range(max(-2, lD - 1), min(2, lD + 1) + 1):
                    nu = lBC - lD
                    ka, sa = d1row[lA][nu]
                    d2n, sgn2 = d2val(lBC, lB2)
                    ddp = p.tt(vBC[ka], d2n)
                    sc = sa * sgn2
                    if lBC == 0:
                        h = z(H1_1[(lBC, lD)]) * h2
                        if h != 0j:
                            const_terms.append((h * sc, ddp))
                    elif (lBC, lD) in cU:
                        ev_terms.append((sc, cU[(lBC, lD)], ddp))

                def wsum(comp):
                    node = None
                    for (sc, (ur, ui), ddp) in ev_terms:
                        tr = p.tt(ur, ddp)
                        ti = p.tt(ui, ddp)
                        cfr = sc * (h2r if comp == "re" else h2i)
                        cfi = sc * (-h2i if comp == "re" else h2r)
                        if cfr != 0.0:
                            node = p.ts(tr, cfr) if node is None else p.stt(tr, cfr, node)
                        if cfi != 0.0:
                            node = p.ts(ti, cfi) if node is None else p.stt(ti, cfi, node)
                    for (cf0, ddp) in const_terms:
                        cf = float(cf0.real) if comp == "re" else float(cf0.imag)
                        if cf == 0.0:
                            continue
                        node = p.ts(ddp, cf) if node is None else p.stt(ddp, cf, node)
                    return node
                wre, wim = wsum("re"), wsum("im")
                if wre is None and wim is None:
                    continue
                W[(lA, lD, lB2)] = (wre, wim)

    Zc = {}
    for key, (wre, wim) in W.items():
        lA, lD, lB2 = key
        Pr, Pi = P1[lB2]
        rr = p.tt(Pr, wre); ii = p.tt(Pi, wim)
        ri = p.tt(Pr, wim); ir = p.tt(Pi, wre)
        Zc[key] = (p.tt(rr, ii, SUB), p.tt(ri, ir, ADD))

    for lA in (-1, 1):
        for lB in (-1, 0, 1):
            for lD in (-1, 0, 1):
                acc = amps.get((lA, lB, lD))
                for lB2 in (-1, 1):
                    if (lA, lD, lB2) not in Zc:
                        continue
                    zr, zi = Zc[(lA, lD, lB2)]
                    ka, sa = d1row[lB2][lB]
                    val = vT1[ka]
                    tre = p.tt(val, zr); tim = p.tt(val, zi)
                    if acc is None:
                        acc = ((p.ts(tre, sa), p.ts(tim, sa)) if sa != 1.0 else (tre, tim))
                    else:
                        op = ADD if sa > 0 else SUB
                        acc = (p.tt(acc[0], tre, op), p.tt(acc[1], tim, op))
                amps[(lA, lB, lD)] = acc

    # ---- res2 ----
    b2r, b2i = bw2
    u1 = p.tt(b2r, cB_); u2 = p.tt(b2i, sB_); u3 = p.tt(b2i, cB_); u4 = p.tt(b2r, sB_)
    P2 = {1: (p.tt(u1, u2, ADD), p.tt(u3, u4, SUB)),
          -1: (p.tt(u1, u2, SUB), p.tt(u3, u4, ADD)),
          0: bw2}

    cU2 = {}
    for lCD in (-1, 1):
        for lB in (-1, 0, 1):
            if abs(lCD - lB) > 1:
                continue
            h = z(H1_2[(lCD, lB)])
            if h == 0j:
                continue
            hr, hi = float(h.real), float(h.imag)
            if lCD == 1:
                re = p.stt(sC1, hi, p.ts(cC1, hr))
                im = p.stt(sC1, -hr, p.ts(cC1, hi))
            else:
                re = p.stt(sC1, -hi, p.ts(cC1, hr))
                im = p.stt(sC1, hr, p.ts(cC1, hi))
            cU2[(lCD, lB)] = (re, im)

    V = {}
    for lA in (-1, 1):
        for lB in (-1, 0, 1):
            for lC2 in (-1, 0, 1):
                h2 = z(H2_2[lC2])
                if h2 == 0j:
                    continue
                h2r, h2i = float(h2.real), float(h2.imag)
                ev_terms, const_terms = [], []
                for lCD in (-1, 0, 1):
                    if abs(lCD - lB) > 1:
                        continue
                    h1 = z(H1_2[(lCD, lB)])
                    if h1 is None or h1 == 0j:
                        continue
                    nu = lCD - lB
                    ka, sa = d1row[lA][nu]
                    kb, sb = d1row[lCD][lC2]
                    ddp = vprod(vCD, ka, vCC, kb, "r2")
                    if lCD == 0:
                        const_terms.append((h1 * h2 * sa * sb, ddp))
                    else:
                        ev_terms.append((sa * sb, cU2[(lCD, lB)], ddp))

                def vsum(comp):
                    node = None
                    for (sc, (ur, ui), ddp) in ev_terms:
                        tr = p.tt(ur, ddp)
                        ti = p.tt(ui, ddp)
                        cfr = sc * (h2r if comp == "re" else h2i)
                        cfi = sc * (-h2i if comp == "re" else h2r)
                        if cfr != 0.0:
                            node = p.ts(tr, cfr) if node is None else p.stt(tr, cfr, node)
                        if cfi != 0.0:
                            node = p.ts(ti, cfi) if node is None else p.stt(ti, cfi, node)
                    for (cf0, ddp) in const_terms:
                        cf = float(cf0.real) if comp == "re" else float(cf0.imag)
                        if cf == 0.0:
                            continue
                        node = p.ts(ddp, cf) if node is None else p.stt(ddp, cf, node)
                    return node
                vre, vim = vsum("re"), vsum("im")
                if vre is None and vim is None:
                    continue
                V[(lA, lB, lC2)] = (vre, vim)

    Z2 = {}
    for key, (vre, vim) in V.items():
        lA, lB, lC2 = key
        Pr, Pi = P2[lC2]
        rr = p.tt(Pr, vre); ii = p.tt(Pi, vim)
        ri = p.tt(Pr, vim); ir = p.tt(Pi, vre)
        Z2[key] = (p.tt(rr, ii, SUB), p.tt(ri, ir, ADD))

    for lA in (-1, 1):
        for lB in (-1, 0, 1):
            for lD in (-1, 0, 1):
                acc = amps.get((lA, lB, lD))
                for lC2 in (-1, 0, 1):
                    if (lA, lB, lC2) not in Z2:
                        continue
                    zr, zi = Z2[(lA, lB, lC2)]
                    ka, sa = d1row[lC2][lD]
                    val = vT2[ka]
                    tre = p.tt(val, zr); tim = p.tt(val, zi)
                    if acc is None:
                        acc = ((p.ts(tre, sa), p.ts(tim, sa)) if sa != 1.0 else (tre, tim))
                    else:
                        op = ADD if sa > 0 else SUB
                        acc = (p.tt(acc[0], tre, op), p.tt(acc[1], tim, op))
                amps[(lA, lB, lD)] = acc

    total = None
    final = set()
    for key in sorted(amps):
        ar, ai = amps[key]
        s1 = p.tt(ar, ar)
        final.add(s1)
        total = s1 if total is None else p.tt(total, s1, ADD)
        final.add(total)
        s2 = p.tt(ai, ai)
        final.add(s2)
        total = p.tt(total, s2, ADD)
        final.add(total)
    p.final_chain = final
    return p, total


# ---------------------------------------------------------------- codegen
def emit_bass(params):
    import concourse.bass as bass
    import concourse.tile as tile
    from concourse import bacc, mybir

    prog, out_idx = build_program(params)
    live = prog.live_set([out_idx])
    order = [i for i in range(len(prog.nodes)) if i in live]
    last_use = {}
    for i in order:
        for d in prog.deps(i):
            last_use[d] = i
    last_use[out_idx] = len(prog.nodes) + 1

    # max live for pool sizing
    alive, maxlive = set(), 0
    for i in order:
        alive.add(i)
        for d in prog.deps(i):
            if d in alive and last_use.get(d) == i:
                alive.discard(d)
        maxlive = max(maxlive, len(alive))
    nbufs = maxlive + 10
    assert nbufs * 512 * 4 <= 222 * 1024, nbufs

    nc = bacc.Bacc("TRN2", target_bir_lowering=False, debug=False,
                   num_devices=N_CORES)
    in_aps = {k: nc.dram_tensor(k, [128, COLS], mybir.dt.float32,
                                kind="ExternalInput").ap() for k in INPUT_NAMES}
    out_ap = nc.dram_tensor("total", [128, COLS], mybir.dt.float32,
                            kind="ExternalOutput").ap()

    # const APs for activation biases (pi/2 for cos; every TS add/bias we
    # route to the scalar engine needs its own const tile)
    bias_vals = {HALF_PI}
    for i in order:
        nd = prog.nodes[i]
        if nd[0] == "TS":
            _, a, s1, op0, s2, op1 = nd
            if op0 == ADD and s2 is None:
                bias_vals.add(float(s1))
            elif op0 == MULT and s2 is not None and op1 == ADD:
                bias_vals.add(float(s2))
    for value in sorted(bias_vals):
        key = (mybir.dt.float32, value)
        if key not in nc.const_aps.aps:
            t = nc.alloc_sbuf_tensor(f"constb-{len(nc.const_aps.aps)}", [128, 1], mybir.dt.float32)
            nc.gpsimd.memset(t.ap(), value)
            nc.const_aps.aps[key] = t.ap()
    nc.all_engine_barrier()

    ALU = {MULT: mybir.AluOpType.mult, ADD: mybir.AluOpType.add,
           SUB: mybir.AluOpType.subtract, MAX: mybir.AluOpType.max}
    AFUNC = {"Sin": mybir.ActivationFunctionType.Sin,
             "Sqrt": mybir.ActivationFunctionType.Sqrt}

    use_bf16 = bool(os.environ.get("KERNEL_BF16"))
    region = getattr(prog, "region_start", None)
    finalset = getattr(prog, "final_chain", set())

    def node_dtype(i):
        nd = prog.nodes[i]
        if (not use_bf16 or region is None or i < region or i in finalset
                or nd[0] in ("IN", "REC")):
            return mybir.dt.float32
        return mybir.dt.bfloat16

    with tile.TileContext(nc) as tc:
        with ExitStack() as ctx:
            pool = ctx.enter_context(tc.tile_pool(name="slab", bufs=nbufs))
            for (c0, fd) in COL_TILES:
                tiles = {}
                casts = {}

                def get_op(j, want):
                    """operand j as dtype `want` (casts via idle scalar engine)."""
                    have = node_dtype(j)
                    if have == want:
                        return tiles[j]
                    key = (j, want)
                    if key not in casts:
                        ct = pool.tile([128, fd], want, tag="slab")
                        nc.scalar.activation(ct[:], tiles[j][:],
                                             mybir.ActivationFunctionType.Copy,
                                             bias=0.0, scale=1.0)
                        casts[key] = ct
                    return casts[key]

                for i in order:
                    nd = prog.nodes[i]
                    op = nd[0]
                    dt_i = node_dtype(i)
                    t = pool.tile([128, fd], dt_i, tag="slab")
                    if op == "IN":
                        nc.sync.dma_start(t[:], in_aps[nd[1]][:, c0:c0 + fd])
                    elif op == "TT":
                        if nd[1] == nd[2] and nd[3] == MULT:
                            nc.scalar.activation(t[:], tiles[nd[1]][:],
                                                 mybir.ActivationFunctionType.Square,
                                                 bias=0.0, scale=1.0)
                        else:
                            nc.vector.tensor_tensor(t[:], get_op(nd[1], dt_i)[:],
                                                    get_op(nd[2], dt_i)[:], ALU[nd[3]])
                    elif op == "TS":
                        _, a, s1, op0, s2, op1 = nd
                        # scalar-engine offload: the vector engine is the
                        # bottleneck and ACT sits idle (native dtype convert).
                        if s2 is None and op0 == MULT:
                            nc.scalar.activation(t[:], tiles[a][:],
                                                 mybir.ActivationFunctionType.Copy,
                                                 bias=0.0, scale=s1)
                        elif s2 is None and op0 == ADD:
                            nc.scalar.activation(t[:], tiles[a][:],
                                                 mybir.ActivationFunctionType.Identity,
                                                 bias=s1, scale=1.0)
                        elif op0 == MULT and op1 == ADD:
                            nc.scalar.activation(t[:], tiles[a][:],
                                                 mybir.ActivationFunctionType.Identity,
                                                 bias=s2, scale=s1)
                        else:
                            nc.vector.tensor_scalar(t[:], get_op(a, dt_i)[:], s1, s2, ALU[op0], ALU[op1])
                    elif op == "STT":
                        _, a, s, b, op0, op1 = nd
                        nc.vector.scalar_tensor_tensor(t[:], get_op(a, dt_i)[:], s,
                                                       get_op(b, dt_i)[:], ALU[op0], ALU[op1])
                    elif op == "ACT":
                        _, func, a, scale, bias = nd
                        nc.scalar.activation(t[:], tiles[a][:], AFUNC[func], bias=bias, scale=scale)
                    elif op == "REC":
                        nc.vector.reciprocal(t[:], get_op(nd[1], mybir.dt.float32)[:])
                    tiles[i] = t
                    if i == out_idx:
                        nc.sync.dma_start(out_ap[:, c0:c0 + fd], t[:])
    nc.compile()
    return nc


# ---------------------------------------------------------------- kernel
def kernel(**inputs):
    params = np.asarray(inputs["params"], np.float32)
    nc = emit_bass(params)

    from concourse.bass_utils import run_bass_kernel_spmd

    in_maps = []
    for core in range(N_CORES):
        m = {}
        for k in INPUT_NAMES:
            arr = np.asarray(inputs[k], np.float32)[core * PER_CORE:(core + 1) * PER_CORE]
            pad = np.empty(PER_CORE_PAD, np.float32)
            pad[:PER_CORE] = arr
            pad[PER_CORE:] = arr[0]
            m[k] = pad.reshape(128, COLS)
        in_maps.append(m)

    trace = bool(os.environ.get("KERNEL_TRACE"))
    if trace:
        try:
            import sys, types
            from trn_agent_boot.trn_boot import _ntff_profile_via_ctypes
            hook = _ntff_profile_via_ctypes('/opt/axon/libaxon_pjrt.so')
            mmod = types.ModuleType('antenv.axon_hooks')
            mmod.get_axon_ntff_profile_hook = lambda: hook
            mmod.set_axon_ntff_profile_hook = lambda h: None
            sys.modules.setdefault('antenv.axon_hooks', mmod)
        except Exception as e:
            print("profile shim failed:", e)
            trace = False

    res = run_bass_kernel_spmd(nc, in_maps, core_ids=list(range(N_CORES)),
                               trace=trace)
    if trace and res.exec_time_ns is not None:
        print(f"HW exec time: {res.exec_time_ns} ns")

    out = np.empty(N_FULL, np.float32)
    for core in range(N_CORES):
        out[core * PER_CORE:(core + 1) * PER_CORE] = \
            res.results[core]["total"].reshape(-1)[:PER_CORE]
    return out
